# revision 17
# baseline (speedup 1.0000x reference)
"""Trainium2 Bass kernel for dual-softmax cosine-similarity attention.

Per batch b:
    pn = p / ||p||,  qn = q / ||q||           (L2 over D)
    S  = pn @ qn^T                            [L, L]
    out_p = softmax(S, axis=1) @ q            [L, D]
    out_q = softmax(S, axis=0) @ p            [L, D]

Shapes: B=64, L=512, D=768 fp32. Data-parallel over B across 8 cores
(8 batches per core).

All three L x L x D matmuls run on the PE in fp8-e4m3 DoubleRow mode
(2x bf16 MAC rate, K=256 per instruction):
  - similarity: host ships exactly-normalized pn^T, qn^T scaled x16 in
    fp8; G = 256*cos, logit error ~1e-3.
  - output matmuls use mean-subtracted weights (zero-point trick):
    E = exp(S) = 1 + delta with |delta| <~ 0.2, so delta quantizes to
    fp8 with ~100x less absolute error than E itself:
      dp8 = 64*(E^T - 1)        (DVE, fp8)
      dq8 = 2048*E^T/colsum - 4 (DVE, fp8; colsum from ACT exp accum)
      ps_p = dp8.T @ [q8 | 1] = 64*(out_p_unnorm - colq), ones col
             gives the rowsum; ps_q = dq8.T @ p8 = 2048*out_q - 4*colp
    The rank-one zero-point corrections colq (x) rrec and colp/512 are
    added on the host (colq/colp are plain column sums of the inputs;
    rrec = 1/rowsum ships back as a tiny [L] tensor per batch).
Softmax max-subtraction is skipped: logits are cosines in [-1,1].

All HBM tensors are packed partition-major on the host so every DMA is
one fully-contiguous 3-6KB run per partition (short descriptor runs
halve the effective HBM rate), one DMA per tensor per batch.
"""

import numpy as np
import ml_dtypes

B, L, D = 64, 512, 768
N_CORES = 8
BPC = B // N_CORES  # batches per core
LT = L // 128  # 4
DT = D // 128  # 6
QW = D + 1  # q tile width including the ones column
QP = D + 4  # padded row stride for the q tile

_cache = {}


def _build(bpc=BPC):
    import concourse.tile as tile
    import concourse.mybir as mybir
    from concourse import bacc

    f32 = mybir.dt.float32
    f16 = mybir.dt.float16
    f8 = mybir.dt.float8e4
    AF = mybir.ActivationFunctionType
    DR = mybir.MatmulPerfMode.DoubleRow
    MUL = mybir.AluOpType.mult
    ADD = mybir.AluOpType.add

    nc = bacc.Bacc("TRN2", target_bir_lowering=False, debug=False)

    p_t = nc.dram_tensor("p_t", [bpc, 128, DT, L], f8, kind="ExternalInput").ap()
    q_t = nc.dram_tensor("q_t", [bpc, 128, DT, L], f8, kind="ExternalInput").ap()
    p8d = nc.dram_tensor("p8", [bpc, 128, LT, D], f8, kind="ExternalInput").ap()
    q8d = nc.dram_tensor("q8", [bpc, 128, LT, QP], f8, kind="ExternalInput").ap()
    out_p = nc.dram_tensor("out_p", [bpc, 128, LT, QW], f16, kind="ExternalOutput").ap()
    out_q = nc.dram_tensor("out_q", [bpc, 128, LT, D], f16, kind="ExternalOutput").ap()

    with tile.TileContext(nc) as tc:
        with (
            tc.tile_pool(name="inp", bufs=4) as inp,
            tc.tile_pool(name="ew", bufs=2) as ew,
            tc.tile_pool(name="small", bufs=2) as small,
            tc.tile_pool(name="outs", bufs=2) as outs,
            tc.tile_pool(name="g_ps", bufs=2, space="PSUM") as g_ps,
            tc.tile_pool(name="o_ps", bufs=3, space="PSUM") as o_ps,
        ):
            state = {}

            def emit_load(b):
                pt = inp.tile([128, DT, L], f8, tag="pt", name=f"pt{b}")
                qt = inp.tile([128, DT, L], f8, tag="qt", name=f"qt{b}")
                p8 = inp.tile([128, LT, D], f8, tag="p8", name=f"p8_{b}")
                q8 = inp.tile([128, LT, QP], f8, tag="q8", name=f"q8_{b}")
                nc.sync.dma_start(pt, p_t[b])
                nc.sync.dma_start(qt, q_t[b])
                nc.sync.dma_start(q8, q8d[b])
                nc.sync.dma_start(p8, p8d[b])
                state[b] = dict(pt=pt, qt=qt, p8=p8, q8=q8)

            def emit_g(b):
                """Similarity matmuls (fp8 DoubleRow), exp + colsum, then
                the fp8 mean-subtracted weight tiles dp8/dq8."""
                st = state[b]
                pt, qt = st["pt"], st["qt"]
                et = ew.tile([128, LT, L], f16, tag="et", name=f"et{b}")
                dp8 = ew.tile([128, LT, L], f8, tag="dp8", name=f"dp{b}")
                dq8 = ew.tile([128, LT, L], f8, tag="dq8", name=f"dq{b}")
                colsum = small.tile([128, LT], f32, tag="colsum", name=f"cs{b}")
                rcol2 = small.tile([128, LT], f32, tag="rcol2", name=f"rc{b}")
                for jt in range(LT):
                    gp = g_ps.tile([128, L], f32, tag="g", name=f"g{b}_{jt}")
                    for kp in range(DT // 2):
                        nc.tensor.matmul(
                            gp,
                            lhsT=qt[:, 2 * kp : 2 * kp + 2, jt * 128 : (jt + 1) * 128],
                            rhs=pt[:, 2 * kp : 2 * kp + 2, :],
                            start=(kp == 0),
                            stop=(kp == DT // 2 - 1),
                            perf_mode=DR,
                        )
                    nc.scalar.activation(
                        et[:, jt, :],
                        gp,
                        AF.Exp,
                        scale=1.0 / 256.0,
                        accum_out=colsum[:, jt : jt + 1],
                    )
                    # per-jt so the dq8 chain never waits on later exps
                    nc.vector.tensor_scalar(
                        dp8[:, jt, :], et[:, jt, :], 64.0, -64.0, MUL, ADD
                    )
                    nc.vector.reciprocal(
                        rcol2[:, jt : jt + 1], colsum[:, jt : jt + 1]
                    )
                    nc.vector.tensor_scalar_mul(
                        rcol2[:, jt : jt + 1], rcol2[:, jt : jt + 1], 2048.0
                    )
                    nc.vector.tensor_scalar(
                        dq8[:, jt, :],
                        et[:, jt, :],
                        rcol2[:, jt : jt + 1],
                        -4.0,
                        MUL,
                        ADD,
                    )
                st["dp8"] = dp8
                st["dq8"] = dq8

            def emit_out(b):
                st = state[b]
                p8, q8, dp8, dq8 = st["p8"], st["q8"], st["dp8"], st["dq8"]
                obp = outs.tile([128, LT, QW], f16, tag="obp", name=f"obp{b}")
                obq = outs.tile([128, LT, D], f16, tag="obq", name=f"obq{b}")
                for m in range(LT):
                    mm = slice(m * 128, (m + 1) * 128)
                    # out_p: dp8.T @ [q8 | 1]; 64*rowsum - 32768 in column D
                    ps = o_ps.tile([128, QW], f32, tag="ops", name=f"op{b}_{m}")
                    for u in range(2):
                        nc.tensor.matmul(
                            ps[:, 0:512],
                            lhsT=dp8[:, 2 * u : 2 * u + 2, mm],
                            rhs=q8[:, 2 * u : 2 * u + 2, 0:512],
                            start=(u == 0),
                            stop=(u == 1),
                            perf_mode=DR,
                        )
                    for u in range(2):
                        nc.tensor.matmul(
                            ps[:, 512:QW],
                            lhsT=dp8[:, 2 * u : 2 * u + 2, mm],
                            rhs=q8[:, 2 * u : 2 * u + 2, 512:QW],
                            start=(u == 0),
                            stop=(u == 1),
                            perf_mode=DR,
                        )
                    # ship raw psum (incl. the 64*(rowsum-512) column); the
                    # host applies 1/rowsum together with the rank-one add
                    nc.scalar.copy(obp[:, m, :], ps[:, 0:QW])
                    # out_q: dq8.T @ p8
                    ps2 = o_ps.tile([128, QW], f32, tag="ops", name=f"oq{b}_{m}")
                    for u in range(2):
                        nc.tensor.matmul(
                            ps2[:, 0:512],
                            lhsT=dq8[:, 2 * u : 2 * u + 2, mm],
                            rhs=p8[:, 2 * u : 2 * u + 2, 0:512],
                            start=(u == 0),
                            stop=(u == 1),
                            perf_mode=DR,
                        )
                    for u in range(2):
                        nc.tensor.matmul(
                            ps2[:, 512:D],
                            lhsT=dq8[:, 2 * u : 2 * u + 2, mm],
                            rhs=p8[:, 2 * u : 2 * u + 2, 512:D],
                            start=(u == 0),
                            stop=(u == 1),
                            perf_mode=DR,
                        )
                    nc.vector.tensor_scalar_mul(
                        obq[:, m, :], ps2[:, 0:D], 1.0 / 2048.0
                    )
                    if m % 2 == 1:
                        h = slice(m - 1, m + 1)
                        nc.sync.dma_start(out_p[b, :, h, :], obp[:, h, :])
                        nc.sync.dma_start(out_q[b, :, h, :], obq[:, h, :])

            # Software pipeline: PE stream per step b is
            #   G-matmuls(b) | out-matmuls(b-1)
            # so the exp + dp8/dq8 chain of batch b runs on ACT/DVE while
            # the PE executes out(b-1). Loads run 2-3 batches ahead.
            emit_load(0)
            emit_g(0)
            emit_load(1)
            emit_load(2)
            emit_load(3)
            for b in range(1, bpc):
                emit_g(b)
                emit_out(b - 1)
                if b + 3 < bpc and b + 3 > 3:
                    emit_load(b + 3)
            emit_out(bpc - 1)

    nc.compile()
    return nc


def _get_nc():
    if "nc" not in _cache:
        _cache["nc"] = _build()
    return _cache["nc"]


def _pack(x):
    """[B, L, W] -> partition-major [B, 128, LT, W]."""
    b, l, w = x.shape
    return np.ascontiguousarray(x.reshape(b, l // 128, 128, w).transpose(0, 2, 1, 3))


def _unpack(x):
    """partition-major [B, 128, LT, W] -> [B, L, W]."""
    b, p, lt, w = x.shape
    return x.transpose(0, 2, 1, 3).reshape(b, lt * p, w)


def kernel(p, q):
    from concourse.bass_utils import run_bass_kernel_spmd

    nc = _get_nc()
    p = np.asarray(p, dtype=np.float32)
    q = np.asarray(q, dtype=np.float32)

    # host-side layout/precision prep
    pn = p / np.linalg.norm(p, axis=-1, keepdims=True)
    qn = q / np.linalg.norm(q, axis=-1, keepdims=True)
    f8 = ml_dtypes.float8_e4m3
    pt8 = _pack(np.ascontiguousarray((pn * 16.0).transpose(0, 2, 1)).astype(f8))
    qt8 = _pack(np.ascontiguousarray((qn * 16.0).transpose(0, 2, 1)).astype(f8))
    p8 = _pack(p.astype(f8))
    q8w = np.ones((B, L, QP), dtype=f8)
    q8w[:, :, 0:D] = q.astype(f8)
    q8 = _pack(q8w)
    colq = q.sum(axis=1)  # [B, D]
    colp = p.sum(axis=1)

    in_maps = []
    for c in range(N_CORES):
        sl = slice(c * BPC, (c + 1) * BPC)
        in_maps.append(
            {"p_t": pt8[sl], "q_t": qt8[sl], "p8": p8[sl], "q8": q8[sl]}
        )

    res = run_bass_kernel_spmd(nc, in_maps, core_ids=list(range(N_CORES)))
    _cache["last_result"] = res
    sb_p = _unpack(
        np.concatenate([r["out_p"] for r in res.results], axis=0)
    ).astype(np.float32)
    sb_q = _unpack(
        np.concatenate([r["out_q"] for r in res.results], axis=0)
    ).astype(np.float32)
    # rank-one zero-point corrections + row-softmax normalization
    # sb_p[..., :D] = 64*(out_p_unnorm - colq); sb_p[..., D] = 64*(rowsum-512)
    rrec = 1.0 / (512.0 + sb_p[:, :, D] / 64.0)
    vec_att_p = (sb_p[:, :, 0:D] / 64.0 + colq[:, None, :]) * rrec[:, :, None]
    vec_att_q = sb_q + colp[:, None, :] / 512.0
    return vec_att_p, vec_att_q


if __name__ == "__main__":
    rng = np.random.default_rng(0)
    p = rng.standard_normal((B, L, D)).astype(np.float32)
    q = rng.standard_normal((B, L, D)).astype(np.float32)
    op, oq = kernel(p, q)
    print("shapes:", op.shape, oq.shape, op.dtype, oq.dtype)


# revision 19
# speedup vs baseline: 1.2003x; 1.2003x over previous
"""Trainium2 Bass kernel for dual-softmax cosine-similarity attention.

Per batch b:
    pn = p / ||p||,  qn = q / ||q||           (L2 over D)
    S  = pn @ qn^T                            [L, L]
    out_p = softmax(S, axis=1) @ q            [L, D]
    out_q = softmax(S, axis=0) @ p            [L, D]

Shapes: B=64, L=512, D=768 fp32. Data-parallel over B across 8 cores
(8 batches per core).

All three L x L x D matmuls run on the PE in fp8-e4m3 DoubleRow mode
(2x bf16 MAC rate, K=256 per instruction):
  - similarity: host ships exactly-normalized pn^T, qn^T scaled x16 in
    fp8; G = 256*cos, logit error ~1e-3.
  - output matmuls use mean-subtracted weights (zero-point trick):
    E = exp(S) = 1 + delta with |delta| <~ 0.2, so delta quantizes to
    fp8 with ~100x less absolute error than E itself:
      dp8 = 64*(E^T - 1)        (DVE, fp8)
      dq8 = 2048*E^T/colsum - 4 (DVE, fp8; colsum from ACT exp accum)
      ps_p = dp8.T @ [q8 | 1] = 64*(out_p_unnorm - colq), ones col
             gives the rowsum; ps_q = dq8.T @ p8 = 2048*out_q - 4*colp
    The rank-one zero-point corrections colq (x) rrec and colp/512 are
    added on the host (colq/colp are plain column sums of the inputs;
    rrec = 1/rowsum ships back as a tiny [L] tensor per batch).
Softmax max-subtraction is skipped: logits are cosines in [-1,1].

All HBM tensors are packed partition-major on the host so every DMA is
one fully-contiguous 3-6KB run per partition (short descriptor runs
halve the effective HBM rate), one DMA per tensor per batch.
"""

import numpy as np
import ml_dtypes

B, L, D = 64, 512, 768
N_CORES = 8
BPC = B // N_CORES  # batches per core
LT = L // 128  # 4
DT = D // 128  # 6
QW = D + 1  # q tile width including the ones column
QP = D + 4  # padded row stride for the q tile

_cache = {}


def _build(bpc=BPC):
    import concourse.tile as tile
    import concourse.mybir as mybir
    from concourse import bacc

    f32 = mybir.dt.float32
    f16 = mybir.dt.float16
    f8 = mybir.dt.float8e4
    AF = mybir.ActivationFunctionType
    DR = mybir.MatmulPerfMode.DoubleRow
    MUL = mybir.AluOpType.mult
    ADD = mybir.AluOpType.add

    nc = bacc.Bacc("TRN2", target_bir_lowering=False, debug=False)

    p_t = nc.dram_tensor("p_t", [bpc, 128, DT, L], f8, kind="ExternalInput").ap()
    q_t = nc.dram_tensor("q_t", [bpc, 128, DT, L], f8, kind="ExternalInput").ap()
    p8d = nc.dram_tensor("p8", [bpc, 128, LT, D], f8, kind="ExternalInput").ap()
    q8d = nc.dram_tensor("q8", [bpc, 128, LT, QP], f8, kind="ExternalInput").ap()
    out_p = nc.dram_tensor("out_p", [bpc, 128, LT, QW], f16, kind="ExternalOutput").ap()
    out_q = nc.dram_tensor("out_q", [bpc, 128, LT, D], f16, kind="ExternalOutput").ap()

    with tile.TileContext(nc) as tc:
        with (
            tc.tile_pool(name="inp", bufs=4) as inp,
            tc.tile_pool(name="ew", bufs=2) as ew,
            tc.tile_pool(name="small", bufs=2) as small,
            tc.tile_pool(name="outs", bufs=2) as outs,
            tc.tile_pool(name="g_ps", bufs=2, space="PSUM") as g_ps,
            tc.tile_pool(name="o_ps", bufs=3, space="PSUM") as o_ps,
        ):
            state = {}

            def emit_load(b):
                pt = inp.tile([128, DT, L], f8, tag="pt", name=f"pt{b}")
                qt = inp.tile([128, DT, L], f8, tag="qt", name=f"qt{b}")
                p8 = inp.tile([128, LT, D], f8, tag="p8", name=f"p8_{b}")
                q8 = inp.tile([128, LT, QP], f8, tag="q8", name=f"q8_{b}")
                nc.sync.dma_start(pt, p_t[b])
                nc.sync.dma_start(qt, q_t[b])
                nc.sync.dma_start(q8, q8d[b])
                nc.sync.dma_start(p8, p8d[b])
                state[b] = dict(pt=pt, qt=qt, p8=p8, q8=q8)

            def emit_g(b):
                """Similarity matmuls (fp8 DoubleRow), exp + colsum, then
                the fp8 mean-subtracted weight tiles dp8/dq8."""
                st = state[b]
                pt, qt = st["pt"], st["qt"]
                et = ew.tile([128, LT, L], f16, tag="et", name=f"et{b}")
                dp8 = ew.tile([128, LT, L], f8, tag="dp8", name=f"dp{b}")
                dq8 = ew.tile([128, LT, L], f8, tag="dq8", name=f"dq{b}")
                colsum = small.tile([128, LT], f32, tag="colsum", name=f"cs{b}")
                rcol2 = small.tile([128, LT], f32, tag="rcol2", name=f"rc{b}")
                for jt in range(LT):
                    gp = g_ps.tile([128, L], f32, tag="g", name=f"g{b}_{jt}")
                    for kp in range(DT // 2):
                        nc.tensor.matmul(
                            gp,
                            lhsT=qt[:, 2 * kp : 2 * kp + 2, jt * 128 : (jt + 1) * 128],
                            rhs=pt[:, 2 * kp : 2 * kp + 2, :],
                            start=(kp == 0),
                            stop=(kp == DT // 2 - 1),
                            perf_mode=DR,
                        )
                    nc.scalar.activation(
                        et[:, jt, :],
                        gp,
                        AF.Exp,
                        scale=1.0 / 256.0,
                        accum_out=colsum[:, jt : jt + 1],
                    )
                    # per-jt so the dq8 chain never waits on later exps;
                    # dp8 on ACT, dq8 + evacs on DVE (makespan balance)
                    nc.scalar.activation(
                        dp8[:, jt, :], et[:, jt, :], AF.Copy, bias=-64.0, scale=64.0
                    )
                    nc.vector.reciprocal(
                        rcol2[:, jt : jt + 1], colsum[:, jt : jt + 1]
                    )
                    nc.vector.tensor_scalar_mul(
                        rcol2[:, jt : jt + 1], rcol2[:, jt : jt + 1], 2048.0
                    )
                    nc.vector.tensor_scalar(
                        dq8[:, jt, :],
                        et[:, jt, :],
                        rcol2[:, jt : jt + 1],
                        -4.0,
                        MUL,
                        ADD,
                    )
                st["dp8"] = dp8
                st["dq8"] = dq8

            def emit_out(b):
                st = state[b]
                p8, q8, dp8, dq8 = st["p8"], st["q8"], st["dp8"], st["dq8"]
                obp = outs.tile([128, LT, QW], f16, tag="obp", name=f"obp{b}")
                obq = outs.tile([128, LT, D], f16, tag="obq", name=f"obq{b}")
                for m in range(LT):
                    mm = slice(m * 128, (m + 1) * 128)
                    # out_p: dp8.T @ [q8 | 1]; 64*rowsum - 32768 in column D
                    ps = o_ps.tile([128, QW], f32, tag="ops", name=f"op{b}_{m}")
                    for u in range(2):
                        nc.tensor.matmul(
                            ps[:, 0:512],
                            lhsT=dp8[:, 2 * u : 2 * u + 2, mm],
                            rhs=q8[:, 2 * u : 2 * u + 2, 0:512],
                            start=(u == 0),
                            stop=(u == 1),
                            perf_mode=DR,
                        )
                    for u in range(2):
                        nc.tensor.matmul(
                            ps[:, 512:QW],
                            lhsT=dp8[:, 2 * u : 2 * u + 2, mm],
                            rhs=q8[:, 2 * u : 2 * u + 2, 512:QW],
                            start=(u == 0),
                            stop=(u == 1),
                            perf_mode=DR,
                        )
                    # ship raw psum (incl. the 64*(rowsum-512) column); the
                    # host applies 1/rowsum together with the rank-one add
                    nc.vector.tensor_copy(obp[:, m, :], ps[:, 0:QW])
                    # out_q: dq8.T @ p8
                    ps2 = o_ps.tile([128, QW], f32, tag="ops", name=f"oq{b}_{m}")
                    for u in range(2):
                        nc.tensor.matmul(
                            ps2[:, 0:512],
                            lhsT=dq8[:, 2 * u : 2 * u + 2, mm],
                            rhs=p8[:, 2 * u : 2 * u + 2, 0:512],
                            start=(u == 0),
                            stop=(u == 1),
                            perf_mode=DR,
                        )
                    for u in range(2):
                        nc.tensor.matmul(
                            ps2[:, 512:D],
                            lhsT=dq8[:, 2 * u : 2 * u + 2, mm],
                            rhs=p8[:, 2 * u : 2 * u + 2, 512:D],
                            start=(u == 0),
                            stop=(u == 1),
                            perf_mode=DR,
                        )
                    nc.vector.tensor_scalar_mul(
                        obq[:, m, :], ps2[:, 0:D], 1.0 / 2048.0
                    )
                    if m % 2 == 1:
                        h = slice(m - 1, m + 1)
                        nc.sync.dma_start(out_p[b, :, h, :], obp[:, h, :])
                        nc.sync.dma_start(out_q[b, :, h, :], obq[:, h, :])

            # Software pipeline: PE stream per step b is
            #   G-matmuls(b) | out-matmuls(b-1)
            # so the exp + dp8/dq8 chain of batch b runs on ACT/DVE while
            # the PE executes out(b-1). Loads run 2-3 batches ahead.
            emit_load(0)
            emit_g(0)
            emit_load(1)
            emit_load(2)
            emit_load(3)
            for b in range(1, bpc):
                emit_g(b)
                emit_out(b - 1)
                if b + 3 < bpc and b + 3 > 3:
                    emit_load(b + 3)
            emit_out(bpc - 1)

    nc.compile()
    return nc


def _get_nc():
    if "nc" not in _cache:
        _cache["nc"] = _build()
    return _cache["nc"]


def _pack(x):
    """[B, L, W] -> partition-major [B, 128, LT, W]."""
    b, l, w = x.shape
    return np.ascontiguousarray(x.reshape(b, l // 128, 128, w).transpose(0, 2, 1, 3))


def _unpack(x):
    """partition-major [B, 128, LT, W] -> [B, L, W]."""
    b, p, lt, w = x.shape
    return x.transpose(0, 2, 1, 3).reshape(b, lt * p, w)


def kernel(p, q):
    from concourse.bass_utils import run_bass_kernel_spmd

    nc = _get_nc()
    p = np.asarray(p, dtype=np.float32)
    q = np.asarray(q, dtype=np.float32)

    # host-side layout/precision prep
    pn = p / np.linalg.norm(p, axis=-1, keepdims=True)
    qn = q / np.linalg.norm(q, axis=-1, keepdims=True)
    f8 = ml_dtypes.float8_e4m3
    pt8 = _pack(np.ascontiguousarray((pn * 16.0).transpose(0, 2, 1)).astype(f8))
    qt8 = _pack(np.ascontiguousarray((qn * 16.0).transpose(0, 2, 1)).astype(f8))
    p8 = _pack(p.astype(f8))
    q8w = np.ones((B, L, QP), dtype=f8)
    q8w[:, :, 0:D] = q.astype(f8)
    q8 = _pack(q8w)
    colq = q.sum(axis=1)  # [B, D]
    colp = p.sum(axis=1)

    in_maps = []
    for c in range(N_CORES):
        sl = slice(c * BPC, (c + 1) * BPC)
        in_maps.append(
            {"p_t": pt8[sl], "q_t": qt8[sl], "p8": p8[sl], "q8": q8[sl]}
        )

    res = run_bass_kernel_spmd(nc, in_maps, core_ids=list(range(N_CORES)))
    _cache["last_result"] = res
    sb_p = _unpack(
        np.concatenate([r["out_p"] for r in res.results], axis=0)
    ).astype(np.float32)
    sb_q = _unpack(
        np.concatenate([r["out_q"] for r in res.results], axis=0)
    ).astype(np.float32)
    # rank-one zero-point corrections + row-softmax normalization
    # sb_p[..., :D] = 64*(out_p_unnorm - colq); sb_p[..., D] = 64*(rowsum-512)
    rrec = 1.0 / (512.0 + sb_p[:, :, D] / 64.0)
    vec_att_p = (sb_p[:, :, 0:D] / 64.0 + colq[:, None, :]) * rrec[:, :, None]
    vec_att_q = sb_q + colp[:, None, :] / 512.0
    return vec_att_p, vec_att_q


if __name__ == "__main__":
    rng = np.random.default_rng(0)
    p = rng.standard_normal((B, L, D)).astype(np.float32)
    q = rng.standard_normal((B, L, D)).astype(np.float32)
    op, oq = kernel(p, q)
    print("shapes:", op.shape, oq.shape, op.dtype, oq.dtype)
